# revision 14
# baseline (speedup 1.0000x reference)
"""Trainium2 Bass kernel for nn_AggregateClusteredSum.

Data-parallel over the batch axis: 32 rows / 8 NeuronCores = 4 rows per core.
Per row, segment sums of hs over 64 clusters are computed as accumulating
matmuls with on-device one-hot matrices (built 8 chunks at a time by a single
DVE is_equal over zero-stride broadcast views of cs and an iota constant).
The one-hot is the stationary operand (64-wide weight loads), giving a
cluster-major [64, 128] accumulator that is transposed once per row on the
PE. The 6-layer PReLU MLP runs feature-major over all 4*129 tokens at once
(natural weight layout stationary, bias+PReLU fused into the ACT eviction),
followed by the leave-one-out aggregation on DVE and a final PE transpose
per row.

DMA structure: hs streams as four 2-MiB f32->bf16 cast transfers on the
SWDGE queue; everything else (weights, biases, iota, identities, exists
masks, cs in partition-block layout, h_n columns) is packed by the host
into ONE [128, ~3.2K] f32 tensor moved by a single HWDGE transfer, so the
queues never clog with small packets.

Host-side work is limited to index metadata (exists mask, Ks reassignment,
G_mask - pure functions of cs) plus input packing/slicing.
"""
import os
import sys

for _p in ("/opt/trn_rl_repo", "/root/.axon_site/_ro/trn_rl_repo"):
    if os.path.isdir(_p) and _p not in sys.path:
        sys.path.insert(0, _p)

import numpy as np
from contextlib import ExitStack

import concourse.bass as bass
import concourse.tile as tile
from concourse import bacc, mybir
from concourse.bass_utils import run_bass_kernel_spmd

F32 = mybir.dt.float32
F16 = mybir.dt.float16
BF16 = mybir.dt.bfloat16

N_CORES = 8
K = 64                      # clusters
H = 128                     # hidden dim of hs
G_DIM = 128                 # output dim
HID = 256                   # MLP hidden
P = 128                     # partitions
NB = 8                      # one-hot chunks built per DVE op

_PROGRAM_CACHE = {}
LAST_RESULT = None          # BassKernelResults of the most recent run (for profiling)
TRACE = False


def _pack_layout(rows_per_core, nch, rem):
    """Column offsets in the packed params tensors (f32 pack + fp16 wpack)."""
    cs_cols = nch + (1 if rem > 0 else 0)
    off = {}
    c = 0
    off["iota"] = c; c += K
    off["ident"] = c; c += P
    for li in range(1, 6):
        for hi in range(2):
            off[f"b{li}_{hi}"] = c; c += 1
    off["b6"] = c; c += 1
    off["hn"] = c; c += rows_per_core
    off["em"] = c; c += rows_per_core * K
    off["cs"] = c; c += rows_per_core * max(cs_cols, 1)
    off["_total"] = c
    # fp16 pack: weights + eye(64)
    w = {}
    c = 0
    w["w1"] = c; c += HID
    for li in (2, 3, 4, 5):
        for ci in range(2):
            w[f"w{li}_{ci}"] = c; c += HID
    for ci in range(2):
        w[f"w6_{ci}"] = c; c += G_DIM
    w["id16"] = c; c += K            # eye(64) in rows 0:64
    w["_total"] = c
    return off, w


def _build_program(rows_per_core, n, alphas):
    """Build the per-core Bass program. Same program for all cores (SPMD)."""
    nch = n // P            # full 128-row chunks per batch row
    rem = n - nch * P       # remainder rows (0 for n=4096)
    ntok = 2 * K + 1        # 129 tokens per row
    T = rows_per_core * ntok  # total tokens per core (516)
    # token free-dim chunks for the MLP (PSUM bank limit: 512 f32)
    nt = (T + 511) // 512
    base = T // nt
    tchunks = []
    t0 = 0
    for i in range(nt):
        tw = base + (1 if i < T - base * nt else 0)
        tchunks.append((t0, tw))
        t0 += tw
    assert t0 == T

    cs_cols = nch + (1 if rem > 0 else 0)
    off, woff = _pack_layout(rows_per_core, nch, rem)
    PW = off["_total"]
    WW = woff["_total"]

    nc = bacc.Bacc()
    hs_in = nc.declare_dram_parameter("hs4", [rows_per_core, n + 1, H], F32, isOutput=False)
    pk_in = nc.declare_dram_parameter("pack", [P, PW], F32, isOutput=False)
    wp_in = nc.declare_dram_parameter("wpack", [P, WW], F16, isOutput=False)
    g_out = nc.declare_dram_parameter("g4", [rows_per_core, K + 1, G_DIM], F32, isOutput=True)

    a1, a2, a3, a4, a5 = [float(a) for a in alphas]
    Act = mybir.ActivationFunctionType
    Alu = mybir.AluOpType

    with tile.TileContext(nc) as tc, ExitStack() as ctx:
        cpool = ctx.enter_context(tc.tile_pool(name="cpool", bufs=1))
        wpool = ctx.enter_context(tc.tile_pool(name="wpool", bufs=1))
        hspool = ctx.enter_context(tc.tile_pool(name="hspool", bufs=1))
        small = ctx.enter_context(tc.tile_pool(name="small", bufs=2))
        ohpool = ctx.enter_context(tc.tile_pool(name="ohpool", bufs=4))
        xpool = ctx.enter_context(tc.tile_pool(name="xpool", bufs=1))
        loopool = ctx.enter_context(tc.tile_pool(name="loopool", bufs=2))
        pseg = ctx.enter_context(tc.tile_pool(name="pseg", bufs=2, space="PSUM"))
        ptp = ctx.enter_context(tc.tile_pool(name="ptp", bufs=1, space="PSUM"))
        pmlp = ctx.enter_context(tc.tile_pool(name="pmlp", bufs=4, space="PSUM"))

        # ---- hs row loads: split across both DMA paths so HBM fills from
        # t~1us. Rows 0-1: raw f32 on HWDGE + on-chip bf16 convert (DVE/ACT,
        # idle early); rows 2-3: f32->bf16 cast on SWDGE.
        nhw = min(2, rows_per_core)
        half = (nch // 2) * H
        hs_parts = []     # per row: list of (bf16_ap, chunk_base)
        hsf_rows = []
        for r in range(rows_per_core):
            src = hs_in[r, 0:nch * P, :].rearrange("(p c) h -> p (c h)", p=P)
            if r < nhw and nch >= 2:
                hf = hspool.tile([P, nch * H], F32, tag=f"hsf{r}", name=f"hsf_{r}")
                nc.sync.dma_start(out=hf[:], in_=src)
                hsf_rows.append(hf)
                ha = hspool.tile([P, half], BF16, tag=f"hsb{r}a", name=f"hsb_{r}a")
                hb = hspool.tile([P, nch * H - half], BF16, tag=f"hsb{r}b", name=f"hsb_{r}b")
                cvt = nc.vector.tensor_copy if r % 2 == 0 else nc.scalar.copy
                cvt(ha[:], hf[:, 0:half])
                cvt(hb[:], hf[:, half:nch * H])
                hs_parts.append([(ha, 0), (hb, nch // 2)])
            else:
                hp = hspool.tile([P, nch * H], BF16, tag=f"hs{r}", name=f"hs_{r}")
                nc.gpsimd.dma_start(out=hp[:], in_=src)
                hs_parts.append([(hp, 0)])

        # ---- two packed transfers for everything else (HWDGE) ----
        pk = cpool.tile([P, PW], F32)
        nc.sync.dma_start(out=pk[:], in_=pk_in[:])
        wp = wpool.tile([P, WW], F16)
        nc.sync.dma_start(out=wp[:], in_=wp_in[:])

        def pslice(name, w):
            o = off[name]
            return pk[:, o:o + w]

        iota_sb = pslice("iota", K)
        ident_sb = pslice("ident", P)
        b_sb = {(li, hi): pslice(f"b{li}_{hi}", 1)
                for li in range(1, 6) for hi in range(2)}
        b6_sb = pslice("b6", 1)

        w1_sb = wp[:, woff["w1"]:woff["w1"] + HID]
        w_sb = {}
        for li in (2, 3, 4, 5, 6):
            for ci in range(2):
                wdim = HID if li < 6 else G_DIM
                o = woff[f"w{li}_{ci}"]
                w_sb[(li, ci)] = wp[:, o:o + wdim]
        id16_sb = wp[0:K, woff["id16"]:woff["id16"] + K]

        # X0: Hcat^T for all rows, feature-major [H, T] fp16
        x0 = xpool.tile([P, T], F16, tag="x0")

        # ---- Stage A: per-row segment sums -> X0 columns ----
        for r in range(rows_per_core):
            parts = hs_parts[r]

            def hs_chunk(cg):
                for ap_, base in reversed(parts):
                    if cg >= base:
                        cl = cg - base
                        return ap_[:, cl * H:(cl + 1) * H]
                raise AssertionError(cg)

            cs_t = pk[:, off["cs"] + r * max(cs_cols, 1):off["cs"] + (r + 1) * max(cs_cols, 1)]
            psC = pseg.tile([K, P], F32, tag="psC", name=f"psC_{r}")
            last = (rem == 0)
            for b0 in range(0, nch, NB):
                bw = min(NB, nch - b0)
                oh = ohpool.tile([P, NB * K], BF16, tag="oh", name=f"oh_{r}_{b0}")
                cs_b = cs_t[:, b0:b0 + bw].broadcast_to((P, bw, K))
                io_b = iota_sb.unsqueeze(1).broadcast_to((P, bw, K))
                nc.vector.tensor_tensor(
                    oh[:].rearrange("p (c k) -> p c k", k=K)[:, 0:bw, :],
                    cs_b, io_b, Alu.is_equal)
                for cc in range(bw):
                    cg = b0 + cc
                    nc.tensor.matmul(
                        psC[:], oh[:, cc * K:(cc + 1) * K],
                        hs_chunk(cg),
                        start=(cg == 0),
                        stop=(last and cg == nch - 1))
            if rem > 0:
                hs_r = hspool.tile([P, H], BF16, tag="hs_rem")
                nc.gpsimd.dma_start(out=hs_r[0:rem, :], in_=hs_in[r, nch * P:n, :])
                oh_r = ohpool.tile([P, K], BF16, tag="oh_rem")
                nc.vector.tensor_scalar(oh_r[0:rem, :], iota_sb[0:rem, :],
                                        cs_t[0:rem, nch:nch + 1], None, Alu.is_equal)
                nc.tensor.matmul(psC[:], oh_r[0:rem, :], hs_r[0:rem, :],
                                 start=(nch == 0), stop=True)

            # cluster-major [64, 128] -> fp16 -> transpose -> [128, 64]
            cm = small.tile([K, P], F16, tag="cm", name=f"cm_{r}")
            nc.scalar.copy(cm[:], psC[:])
            tps = ptp.tile([P, K], F16, tag="tps", name=f"tps_{r}")
            nc.tensor.transpose(tps[:], cm[:], id16_sb[:])

            r0 = r * ntok
            hn_col = pk[:, off["hn"] + r:off["hn"] + r + 1]     # f32 [128,1]
            nc.scalar.copy(x0[:, r0:r0 + K], tps[:])
            nc.vector.tensor_scalar(x0[:, r0 + K:r0 + 2 * K], tps[:],
                                    hn_col, None, Alu.add)
            nc.vector.tensor_scalar(x0[:, r0 + 2 * K:r0 + 2 * K + 1],
                                    hn_col, 0.0, None, Alu.add)

        # ---- Stage B: MLP over all T tokens, feature-major ----
        # PReLU eviction: ACT for half 0, DVE (3-op max/min/fma) for half 1,
        # so the two halves' evictions run on different engines.
        dve_scr = ctx.enter_context(tc.tile_pool(name="dve_scr", bufs=2))

        def evict_prelu(dst, ps_ap, bias_ap, alpha, h, name):
            nc.scalar.activation(dst, ps_ap, Act.Prelu,
                                 bias=bias_ap, scale=1.0, alpha=alpha)

        x1 = [xpool.tile([P, T], F16, tag=f"x1_{h}", name=f"x1_{h}") for h in range(2)]
        for h in range(2):
            for (t0, tw) in tchunks:
                ps = pmlp.tile([P, tw], F32, tag="pmlp", name=f"ps1_{h}_{t0}")
                nc.tensor.matmul(ps[:], w1_sb[:, h * P:(h + 1) * P], x0[:, t0:t0 + tw],
                                 start=True, stop=True)
                evict_prelu(x1[h][:, t0:t0 + tw], ps[:], b_sb[(1, h)], a1, h, f"1_{h}_{t0}")
        xprev = x1
        for li, alpha in ((2, a2), (3, a3), (4, a4), (5, a5)):
            xn = [xpool.tile([P, T], F16, tag=f"x{li}_{h}", name=f"x{li}_{h}") for h in range(2)]
            for h in range(2):
                # ci outer, token chunk inner: one weight load serves both
                # token chunks (the PE reloads stationary per matmul)
                pss = [pmlp.tile([P, tw], F32, tag="pmlp", name=f"ps{li}_{h}_{t0}")
                       for (t0, tw) in tchunks]
                for ci in range(2):
                    for ti, (t0, tw) in enumerate(tchunks):
                        nc.tensor.matmul(pss[ti][:], w_sb[(li, ci)][:, h * P:(h + 1) * P],
                                         xprev[ci][:, t0:t0 + tw],
                                         start=(ci == 0), stop=(ci == 1))
                for ti, (t0, tw) in enumerate(tchunks):
                    evict_prelu(xn[h][:, t0:t0 + tw], pss[ti][:], b_sb[(li, h)],
                                alpha, h, f"{li}_{h}_{t0}")
            xprev = xn
        # L6: 256 -> 128, bias only, keep f32
        gs = xpool.tile([P, T], F32, tag="gs")
        pss = [pmlp.tile([P, tw], F32, tag="pmlp", name=f"ps6_{t0}")
               for (t0, tw) in tchunks]
        for ci in range(2):
            for ti, (t0, tw) in enumerate(tchunks):
                nc.tensor.matmul(pss[ti][:], w_sb[(6, ci)][:], xprev[ci][:, t0:t0 + tw],
                                 start=(ci == 0), stop=(ci == 1))
        for ti, (t0, tw) in enumerate(tchunks):
            nc.scalar.activation(gs[:, t0:t0 + tw], pss[ti][:], Act.Identity,
                                 bias=b6_sb, scale=1.0)

        # ---- Stage C: leave-one-out per row; outputs packed, one store ----
        osb = loopool.tile([K + 1, rows_per_core * G_DIM], F32, tag="osb")
        for r in range(rows_per_core):
            r0 = r * ntok
            em_sb = pk[:, off["em"] + r * K:off["em"] + (r + 1) * K]
            scr = loopool.tile([P, K], F32, tag="scr", name=f"scr_{r}")
            s_col = loopool.tile([P, 1], F32, tag="scol", name=f"scol_{r}")
            # scr = gs_lo * em ; s = sum_free(scr)  (masked base sum S)
            nc.vector.scalar_tensor_tensor(scr[:], gs[:, r0:r0 + K], 1.0, em_sb,
                                           Alu.mult, Alu.mult, accum_out=s_col[:])
            gout = loopool.tile([P, K + 1], F32, tag="gout", name=f"gout_{r}")
            tmp = loopool.tile([P, K], F32, tag="tmp", name=f"tmp_{r}")
            # tmp = (gs_hi + S) - gs_lo
            nc.vector.scalar_tensor_tensor(tmp[:], gs[:, r0 + K:r0 + 2 * K], s_col[:],
                                           gs[:, r0:r0 + K], Alu.add, Alu.subtract)
            nc.vector.tensor_tensor(gout[:, 0:K], tmp[:], em_sb, Alu.mult)
            nc.vector.tensor_scalar(gout[:, K:K + 1], gs[:, r0 + 2 * K:r0 + 2 * K + 1],
                                    s_col[:], None, Alu.add)
            # transpose [128g, 65] -> [65, 128g]
            tp = ptp.tile([K + 1, P], F32, tag="tp", name=f"tp_{r}")
            nc.tensor.transpose(tp[:], gout[:], ident_sb)
            nc.scalar.copy(osb[:, r * G_DIM:(r + 1) * G_DIM], tp[:])
        nc.sync.dma_start(
            out=g_out[:].rearrange("r k g -> k r g"),
            in_=osb[:].rearrange("k (r g) -> k r g", g=G_DIM))

    nc.finalize()
    return nc


def kernel(**inputs):
    global LAST_RESULT
    hs = np.ascontiguousarray(np.asarray(inputs["hs"], dtype=np.float32))
    cs = np.asarray(inputs["cs"])
    n = int(np.asarray(inputs["n"]))
    B, N, _H = hs.shape
    assert _H == H and B % N_CORES == 0
    rows_per_core = B // N_CORES
    assert n >= 1
    nch = n // P
    npad = nch * P
    rem = n - npad
    cs_cols = nch + (1 if rem > 0 else 0)

    cs_i = cs.astype(np.int64)
    cs_valid = cs_i[:, :n]                      # entries >= n are masked off

    # ---- host-side index metadata ----
    cnt = np.zeros((B, K), dtype=np.int64)
    for b in range(B):
        cnt[b] = np.bincount(cs_valid[b], minlength=K)[:K]
    exists = (cnt > 0).astype(np.float32)       # [B, K]
    Ks = cs_valid.max(axis=1)                   # [B] (cs values are >= 0)

    # ---- packed params tensors per core ----
    off, woff = _pack_layout(rows_per_core, nch, rem)
    PW = off["_total"]
    WW = woff["_total"]

    def wmat(x):
        return np.asarray(x, dtype=np.float32)

    wpack = np.zeros((P, WW), dtype=np.float16)
    wpack[:, woff["w1"]:woff["w1"] + HID] = wmat(inputs["W1"]).astype(np.float16)
    for li in (2, 3, 4, 5):
        w = wmat(inputs[f"W{li}"]).astype(np.float16)
        for ci in range(2):
            o = woff[f"w{li}_{ci}"]
            wpack[:, o:o + HID] = w[ci * P:(ci + 1) * P]
    w6 = wmat(inputs["W6"]).astype(np.float16)
    for ci in range(2):
        o = woff[f"w6_{ci}"]
        wpack[:, o:o + G_DIM] = w6[ci * P:(ci + 1) * P]
    wpack[0:K, woff["id16"]:woff["id16"] + K] = np.eye(K, dtype=np.float16)

    packs = []
    for c in range(N_CORES):
        b0 = c * rows_per_core
        pk = np.zeros((P, PW), dtype=np.float32)
        pk[:, off["iota"]:off["iota"] + K] = np.arange(K, dtype=np.float32)[None, :]
        pk[:, off["ident"]:off["ident"] + P] = np.eye(P, dtype=np.float32)
        for li in range(1, 6):
            b = wmat(inputs[f"b{li}"])
            for hi in range(2):
                pk[:, off[f"b{li}_{hi}"]] = b[hi * P:(hi + 1) * P]
        pk[:, off["b6"]] = wmat(inputs["b6"])
        for r in range(rows_per_core):
            pk[:, off["hn"] + r] = hs[b0 + r, n, :]
            pk[:, off["em"] + r * K:off["em"] + (r + 1) * K] = exists[b0 + r][None, :]
            co = off["cs"] + r * max(cs_cols, 1)
            if nch > 0:
                pk[:, co:co + nch] = cs_valid[b0 + r, :npad].reshape(P, nch)
            if rem > 0:
                pk[:rem, co + nch] = cs_valid[b0 + r, npad:n]
        packs.append(pk)

    alphas = tuple(float(np.asarray(inputs[f"a{i}"])) for i in range(1, 6))
    key = (rows_per_core, n, alphas)
    if key not in _PROGRAM_CACHE:
        _PROGRAM_CACHE[key] = _build_program(rows_per_core, n, alphas)
    nc = _PROGRAM_CACHE[key]

    in_maps = []
    for c in range(N_CORES):
        b0 = c * rows_per_core
        in_maps.append({
            "hs4": np.ascontiguousarray(hs[b0:b0 + rows_per_core, :n + 1, :]),
            "pack": packs[c],
            "wpack": wpack,
        })

    res = run_bass_kernel_spmd(nc, in_maps, list(range(N_CORES)), trace=TRACE)
    LAST_RESULT = res
    G = np.concatenate([r["g4"] for r in res.results], axis=0)  # [B, K+1, G_DIM]

    # ---- host-side Ks reassignment + G_mask (index metadata) ----
    j = np.arange(K + 1)
    small = (Ks <= K - 2)[:, None]
    move = (j[None, :] == (Ks + 1)[:, None]) & small
    G = np.where(move[..., None], G[:, K:K + 1], G)
    G = np.where(((j[None, :] == K) & small)[..., None], np.float32(0.0), G)
    G_mask = np.where((j[None, :] >= (Ks + 2)[:, None]) & small, 0.0, 1.0).astype(np.float32)
    return G.astype(np.float32), G_mask


# revision 16
# speedup vs baseline: 1.1504x; 1.1504x over previous
"""Trainium2 Bass kernel for nn_AggregateClusteredSum.

Data-parallel over the batch axis: 32 rows / 8 NeuronCores = 4 rows per core.
Per row, segment sums of hs over 64 clusters are computed as accumulating
matmuls with on-device one-hot matrices (built 8 chunks at a time by a single
DVE is_equal over zero-stride broadcast views of cs and an iota constant).
The one-hot is the stationary operand (64-wide weight loads), giving a
cluster-major [64, 128] accumulator that is transposed once per row on the
PE. The 6-layer PReLU MLP runs feature-major over all 4*129 tokens at once
(natural weight layout stationary, bias+PReLU fused into the ACT eviction),
followed by the leave-one-out aggregation on DVE and a final PE transpose
per row.

DMA structure: hs streams as four 2-MiB f32->bf16 cast transfers on the
SWDGE queue; everything else (weights, biases, iota, identities, exists
masks, cs in partition-block layout, h_n columns) is packed by the host
into ONE [128, ~3.2K] f32 tensor moved by a single HWDGE transfer, so the
queues never clog with small packets.

Host-side work is limited to index metadata (exists mask, Ks reassignment,
G_mask - pure functions of cs) plus input packing/slicing.
"""
import os
import sys

for _p in ("/opt/trn_rl_repo", "/root/.axon_site/_ro/trn_rl_repo"):
    if os.path.isdir(_p) and _p not in sys.path:
        sys.path.insert(0, _p)

import numpy as np
from contextlib import ExitStack

import concourse.bass as bass
import concourse.tile as tile
from concourse import bacc, mybir
from concourse.bass_utils import run_bass_kernel_spmd

F32 = mybir.dt.float32
F16 = mybir.dt.float16
BF16 = mybir.dt.bfloat16

N_CORES = 8
K = 64                      # clusters
H = 128                     # hidden dim of hs
G_DIM = 128                 # output dim
HID = 256                   # MLP hidden
P = 128                     # partitions
NB = 8                      # one-hot chunks built per DVE op

_PROGRAM_CACHE = {}
LAST_RESULT = None          # BassKernelResults of the most recent run (for profiling)
TRACE = False


def _pack_layout(rows_per_core, nch, rem):
    """Column offsets in the packed params tensors (f32 pack + fp16 wpack)."""
    cs_cols = nch + (1 if rem > 0 else 0)
    off = {}
    c = 0
    off["iota"] = c; c += K
    off["ident"] = c; c += P
    for li in range(1, 6):
        for hi in range(2):
            off[f"b{li}_{hi}"] = c; c += 1
    off["b6"] = c; c += 1
    off["hn"] = c; c += rows_per_core
    off["em"] = c; c += rows_per_core * K
    off["cs"] = c; c += rows_per_core * max(cs_cols, 1)
    off["_total"] = c
    # fp16 pack: weights + eye(64)
    w = {}
    c = 0
    w["w1"] = c; c += HID
    for li in (2, 3, 4, 5):
        for ci in range(2):
            w[f"w{li}_{ci}"] = c; c += HID
    for ci in range(2):
        w[f"w6_{ci}"] = c; c += G_DIM
    w["id16"] = c; c += K            # eye(64) in rows 0:64
    w["_total"] = c
    return off, w


def _build_program(rows_per_core, n, alphas):
    """Build the per-core Bass program. Same program for all cores (SPMD)."""
    nch = n // P            # full 128-row chunks per batch row
    rem = n - nch * P       # remainder rows (0 for n=4096)
    ntok = 2 * K + 1        # 129 tokens per row
    T = rows_per_core * ntok  # total tokens per core (516)
    # token free-dim chunks for the MLP (PSUM bank limit: 512 f32)
    nt = (T + 511) // 512
    base = T // nt
    tchunks = []
    t0 = 0
    for i in range(nt):
        tw = base + (1 if i < T - base * nt else 0)
        tchunks.append((t0, tw))
        t0 += tw
    assert t0 == T

    cs_cols = nch + (1 if rem > 0 else 0)
    off, woff = _pack_layout(rows_per_core, nch, rem)
    PW = off["_total"]
    WW = woff["_total"]

    nc = bacc.Bacc()
    hs_in = nc.declare_dram_parameter("hs4", [rows_per_core, n + 1, H], F32, isOutput=False)
    pk_in = nc.declare_dram_parameter("pack", [P, PW], F32, isOutput=False)
    wp_in = nc.declare_dram_parameter("wpack", [P, WW], F16, isOutput=False)
    g_out = nc.declare_dram_parameter("g4", [rows_per_core, K + 1, G_DIM], F32, isOutput=True)

    a1, a2, a3, a4, a5 = [float(a) for a in alphas]
    Act = mybir.ActivationFunctionType
    Alu = mybir.AluOpType

    with tile.TileContext(nc) as tc, ExitStack() as ctx:
        cpool = ctx.enter_context(tc.tile_pool(name="cpool", bufs=1))
        wpool = ctx.enter_context(tc.tile_pool(name="wpool", bufs=1))
        hspool = ctx.enter_context(tc.tile_pool(name="hspool", bufs=1))
        small = ctx.enter_context(tc.tile_pool(name="small", bufs=2))
        ohpool = ctx.enter_context(tc.tile_pool(name="ohpool", bufs=4))
        xpool = ctx.enter_context(tc.tile_pool(name="xpool", bufs=1))
        loopool = ctx.enter_context(tc.tile_pool(name="loopool", bufs=2))
        pseg = ctx.enter_context(tc.tile_pool(name="pseg", bufs=2, space="PSUM"))
        ptp = ctx.enter_context(tc.tile_pool(name="ptp", bufs=1, space="PSUM"))
        pmlp = ctx.enter_context(tc.tile_pool(name="pmlp", bufs=4, space="PSUM"))

        # ---- hs row loads: f32->bf16 cast on the dedicated SWDGE queue.
        # Row 0 is split in half so its matmuls can start at the halfway
        # point of the first transfer.
        hs_parts = []     # per row: list of (bf16_ap, chunk_base)
        for r in range(rows_per_core):
            src3 = hs_in[r, 0:nch * P, :].rearrange("(p c) h -> p c h", p=P)
            if r == 0 and nch >= 2:
                ch = nch // 2
                ha = hspool.tile([P, ch * H], BF16, tag="hs0a", name="hs_0a")
                nc.gpsimd.dma_start(out=ha[:].rearrange("p (c h) -> p c h", h=H),
                                    in_=src3[:, 0:ch, :])
                hb = hspool.tile([P, (nch - ch) * H], BF16, tag="hs0b", name="hs_0b")
                nc.gpsimd.dma_start(out=hb[:].rearrange("p (c h) -> p c h", h=H),
                                    in_=src3[:, ch:nch, :])
                hs_parts.append([(ha, 0), (hb, ch)])
            else:
                hp = hspool.tile([P, nch * H], BF16, tag=f"hs{r}", name=f"hs_{r}")
                nc.gpsimd.dma_start(
                    out=hp[:], in_=hs_in[r, 0:nch * P, :].rearrange("(p c) h -> p (c h)", p=P))
                hs_parts.append([(hp, 0)])

        # ---- two packed transfers for everything else (HWDGE) ----
        pk = cpool.tile([P, PW], F32)
        nc.sync.dma_start(out=pk[:], in_=pk_in[:])
        wp = wpool.tile([P, WW], F16)
        nc.sync.dma_start(out=wp[:], in_=wp_in[:])

        def pslice(name, w):
            o = off[name]
            return pk[:, o:o + w]

        iota_sb = pslice("iota", K)
        ident_sb = pslice("ident", P)
        b_sb = {(li, hi): pslice(f"b{li}_{hi}", 1)
                for li in range(1, 6) for hi in range(2)}
        b6_sb = pslice("b6", 1)

        w1_sb = wp[:, woff["w1"]:woff["w1"] + HID]
        w_sb = {}
        for li in (2, 3, 4, 5, 6):
            for ci in range(2):
                wdim = HID if li < 6 else G_DIM
                o = woff[f"w{li}_{ci}"]
                w_sb[(li, ci)] = wp[:, o:o + wdim]
        id16_sb = wp[0:K, woff["id16"]:woff["id16"] + K]

        # X0: Hcat^T for all rows, feature-major [H, T] fp16
        x0 = xpool.tile([P, T], F16, tag="x0")
        gs = xpool.tile([P, T], F32, tag="gs")

        def mlp_tokens(sfx, t0, tw):
            """Run the 6-layer MLP on token range [t0, t0+tw) (tw <= 512)."""
            x1 = [xpool.tile([P, tw], F16, tag=f"x1_{h}_{sfx}", name=f"x1_{h}_{sfx}")
                  for h in range(2)]
            for h in range(2):
                ps = pmlp.tile([P, tw], F32, tag="pmlp", name=f"ps1_{h}_{sfx}")
                nc.tensor.matmul(ps[:], w1_sb[:, h * P:(h + 1) * P], x0[:, t0:t0 + tw],
                                 start=True, stop=True)
                nc.scalar.activation(x1[h][:], ps[:], Act.Prelu,
                                     bias=b_sb[(1, h)], scale=1.0, alpha=a1)
            xprev = x1
            for li, alpha in ((2, a2), (3, a3), (4, a4), (5, a5)):
                xn = [xpool.tile([P, tw], F16, tag=f"x{li}_{h}_{sfx}", name=f"x{li}_{h}_{sfx}")
                      for h in range(2)]
                for h in range(2):
                    ps = pmlp.tile([P, tw], F32, tag="pmlp", name=f"ps{li}_{h}_{sfx}")
                    for ci in range(2):
                        nc.tensor.matmul(ps[:], w_sb[(li, ci)][:, h * P:(h + 1) * P],
                                         xprev[ci][:], start=(ci == 0), stop=(ci == 1))
                    nc.scalar.activation(xn[h][:], ps[:], Act.Prelu,
                                         bias=b_sb[(li, h)], scale=1.0, alpha=alpha)
                xprev = xn
            ps = pmlp.tile([P, tw], F32, tag="pmlp", name=f"ps6_{sfx}")
            for ci in range(2):
                nc.tensor.matmul(ps[:], w_sb[(6, ci)][:], xprev[ci][:],
                                 start=(ci == 0), stop=(ci == 1))
            nc.scalar.activation(gs[:, t0:t0 + tw], ps[:], Act.Identity,
                                 bias=b6_sb, scale=1.0)

        rmid = (rows_per_core + 1) // 2

        # ---- Stage A: per-row segment sums -> X0 columns ----
        for r in range(rows_per_core):
            parts = hs_parts[r]

            def hs_chunk(cg):
                for ap_, base in reversed(parts):
                    if cg >= base:
                        cl = cg - base
                        return ap_[:, cl * H:(cl + 1) * H]
                raise AssertionError(cg)

            cs_t = pk[:, off["cs"] + r * max(cs_cols, 1):off["cs"] + (r + 1) * max(cs_cols, 1)]
            psC = pseg.tile([K, P], F32, tag="psC", name=f"psC_{r}")
            last = (rem == 0)
            for b0 in range(0, nch, NB):
                bw = min(NB, nch - b0)
                oh = ohpool.tile([P, NB * K], BF16, tag="oh", name=f"oh_{r}_{b0}")
                cs_b = cs_t[:, b0:b0 + bw].broadcast_to((P, bw, K))
                io_b = iota_sb.unsqueeze(1).broadcast_to((P, bw, K))
                nc.vector.tensor_tensor(
                    oh[:].rearrange("p (c k) -> p c k", k=K)[:, 0:bw, :],
                    cs_b, io_b, Alu.is_equal)
                for cc in range(bw):
                    cg = b0 + cc
                    nc.tensor.matmul(
                        psC[:], oh[:, cc * K:(cc + 1) * K],
                        hs_chunk(cg),
                        start=(cg == 0),
                        stop=(last and cg == nch - 1))
            if rem > 0:
                hs_r = hspool.tile([P, H], BF16, tag="hs_rem")
                nc.gpsimd.dma_start(out=hs_r[0:rem, :], in_=hs_in[r, nch * P:n, :])
                oh_r = ohpool.tile([P, K], BF16, tag="oh_rem")
                nc.vector.tensor_scalar(oh_r[0:rem, :], iota_sb[0:rem, :],
                                        cs_t[0:rem, nch:nch + 1], None, Alu.is_equal)
                nc.tensor.matmul(psC[:], oh_r[0:rem, :], hs_r[0:rem, :],
                                 start=(nch == 0), stop=True)

            # cluster-major [64, 128] -> fp16 -> transpose -> [128, 64]
            cm = small.tile([K, P], F16, tag="cm", name=f"cm_{r}")
            nc.scalar.copy(cm[:], psC[:])
            tps = ptp.tile([P, K], F16, tag="tps", name=f"tps_{r}")
            nc.tensor.transpose(tps[:], cm[:], id16_sb[:])

            r0 = r * ntok
            hn_col = pk[:, off["hn"] + r:off["hn"] + r + 1]     # f32 [128,1]
            nc.scalar.copy(x0[:, r0:r0 + K], tps[:])
            nc.vector.tensor_scalar(x0[:, r0 + K:r0 + 2 * K], tps[:],
                                    hn_col, None, Alu.add)
            nc.vector.tensor_scalar(x0[:, r0 + 2 * K:r0 + 2 * K + 1],
                                    hn_col, 0.0, None, Alu.add)

            if r == rmid - 1:
                mlp_tokens("A", 0, rmid * ntok)
            elif r == rows_per_core - 1 and rows_per_core > rmid:
                mlp_tokens("B", rmid * ntok, (rows_per_core - rmid) * ntok)

        # ---- Stage C: leave-one-out per row; outputs packed, one store ----
        osb = loopool.tile([K + 1, rows_per_core * G_DIM], F32, tag="osb")
        for r in range(rows_per_core):
            r0 = r * ntok
            em_sb = pk[:, off["em"] + r * K:off["em"] + (r + 1) * K]
            scr = loopool.tile([P, K], F32, tag="scr", name=f"scr_{r}")
            s_col = loopool.tile([P, 1], F32, tag="scol", name=f"scol_{r}")
            # scr = gs_lo * em ; s = sum_free(scr)  (masked base sum S)
            nc.vector.scalar_tensor_tensor(scr[:], gs[:, r0:r0 + K], 1.0, em_sb,
                                           Alu.mult, Alu.mult, accum_out=s_col[:])
            gout = loopool.tile([P, K + 1], F32, tag="gout", name=f"gout_{r}")
            tmp = loopool.tile([P, K], F32, tag="tmp", name=f"tmp_{r}")
            # tmp = (gs_hi + S) - gs_lo
            nc.vector.scalar_tensor_tensor(tmp[:], gs[:, r0 + K:r0 + 2 * K], s_col[:],
                                           gs[:, r0:r0 + K], Alu.add, Alu.subtract)
            nc.vector.tensor_tensor(gout[:, 0:K], tmp[:], em_sb, Alu.mult)
            nc.vector.tensor_scalar(gout[:, K:K + 1], gs[:, r0 + 2 * K:r0 + 2 * K + 1],
                                    s_col[:], None, Alu.add)
            # transpose [128g, 65] -> [65, 128g]
            tp = ptp.tile([K + 1, P], F32, tag="tp", name=f"tp_{r}")
            nc.tensor.transpose(tp[:], gout[:], ident_sb)
            nc.scalar.copy(osb[:, r * G_DIM:(r + 1) * G_DIM], tp[:])
        nc.sync.dma_start(
            out=g_out[:].rearrange("r k g -> k r g"),
            in_=osb[:].rearrange("k (r g) -> k r g", g=G_DIM))

    nc.finalize()
    return nc


def kernel(**inputs):
    global LAST_RESULT
    hs = np.ascontiguousarray(np.asarray(inputs["hs"], dtype=np.float32))
    cs = np.asarray(inputs["cs"])
    n = int(np.asarray(inputs["n"]))
    B, N, _H = hs.shape
    assert _H == H and B % N_CORES == 0
    rows_per_core = B // N_CORES
    assert n >= 1
    nch = n // P
    npad = nch * P
    rem = n - npad
    cs_cols = nch + (1 if rem > 0 else 0)

    cs_i = cs.astype(np.int64)
    cs_valid = cs_i[:, :n]                      # entries >= n are masked off

    # ---- host-side index metadata ----
    cnt = np.zeros((B, K), dtype=np.int64)
    for b in range(B):
        cnt[b] = np.bincount(cs_valid[b], minlength=K)[:K]
    exists = (cnt > 0).astype(np.float32)       # [B, K]
    Ks = cs_valid.max(axis=1)                   # [B] (cs values are >= 0)

    # ---- packed params tensors per core ----
    off, woff = _pack_layout(rows_per_core, nch, rem)
    PW = off["_total"]
    WW = woff["_total"]

    def wmat(x):
        return np.asarray(x, dtype=np.float32)

    wpack = np.zeros((P, WW), dtype=np.float16)
    wpack[:, woff["w1"]:woff["w1"] + HID] = wmat(inputs["W1"]).astype(np.float16)
    for li in (2, 3, 4, 5):
        w = wmat(inputs[f"W{li}"]).astype(np.float16)
        for ci in range(2):
            o = woff[f"w{li}_{ci}"]
            wpack[:, o:o + HID] = w[ci * P:(ci + 1) * P]
    w6 = wmat(inputs["W6"]).astype(np.float16)
    for ci in range(2):
        o = woff[f"w6_{ci}"]
        wpack[:, o:o + G_DIM] = w6[ci * P:(ci + 1) * P]
    wpack[0:K, woff["id16"]:woff["id16"] + K] = np.eye(K, dtype=np.float16)

    packs = []
    for c in range(N_CORES):
        b0 = c * rows_per_core
        pk = np.zeros((P, PW), dtype=np.float32)
        pk[:, off["iota"]:off["iota"] + K] = np.arange(K, dtype=np.float32)[None, :]
        pk[:, off["ident"]:off["ident"] + P] = np.eye(P, dtype=np.float32)
        for li in range(1, 6):
            b = wmat(inputs[f"b{li}"])
            for hi in range(2):
                pk[:, off[f"b{li}_{hi}"]] = b[hi * P:(hi + 1) * P]
        pk[:, off["b6"]] = wmat(inputs["b6"])
        for r in range(rows_per_core):
            pk[:, off["hn"] + r] = hs[b0 + r, n, :]
            pk[:, off["em"] + r * K:off["em"] + (r + 1) * K] = exists[b0 + r][None, :]
            co = off["cs"] + r * max(cs_cols, 1)
            if nch > 0:
                pk[:, co:co + nch] = cs_valid[b0 + r, :npad].reshape(P, nch)
            if rem > 0:
                pk[:rem, co + nch] = cs_valid[b0 + r, npad:n]
        packs.append(pk)

    alphas = tuple(float(np.asarray(inputs[f"a{i}"])) for i in range(1, 6))
    key = (rows_per_core, n, alphas)
    if key not in _PROGRAM_CACHE:
        _PROGRAM_CACHE[key] = _build_program(rows_per_core, n, alphas)
    nc = _PROGRAM_CACHE[key]

    in_maps = []
    for c in range(N_CORES):
        b0 = c * rows_per_core
        in_maps.append({
            "hs4": np.ascontiguousarray(hs[b0:b0 + rows_per_core, :n + 1, :]),
            "pack": packs[c],
            "wpack": wpack,
        })

    res = run_bass_kernel_spmd(nc, in_maps, list(range(N_CORES)), trace=TRACE)
    LAST_RESULT = res
    G = np.concatenate([r["g4"] for r in res.results], axis=0)  # [B, K+1, G_DIM]

    # ---- host-side Ks reassignment + G_mask (index metadata) ----
    j = np.arange(K + 1)
    small = (Ks <= K - 2)[:, None]
    move = (j[None, :] == (Ks + 1)[:, None]) & small
    G = np.where(move[..., None], G[:, K:K + 1], G)
    G = np.where(((j[None, :] == K) & small)[..., None], np.float32(0.0), G)
    G_mask = np.where((j[None, :] >= (Ks + 2)[:, None]) & small, 0.0, 1.0).astype(np.float32)
    return G.astype(np.float32), G_mask
